# revision 3
# baseline (speedup 1.0000x reference)
"""Trainium2 kernel for cellpose-style flow integration (grid_sample scan).

Strategy (v2):
  - Host builds a per-cell table T[r*2050+c] = [A_a,A_b,B_a,B_b,C_a,C_b,D_a,D_b]:
    the bilinear patch at padded pixel (r,c) reparameterized around the cell
    center, so on-device sampling is  val = A + tx*B + ty*(C + tx*D)  with
    tx,ty in [-0.5,0.5].  Channels a=x-displacement, b=y-displacement.
    Zero padding rows/cols encode grid_sample's zeros-padding.
  - Points sharded across 8 NeuronCores (32768 each, laid out [128,256]),
    processed in 4 column-chunks of 64 forming 4 independent software
    pipelines so SWDGE descriptor generation (the bottleneck: 128-offset
    indirect DMAs, ~1us each on the frozen Q7 pair) never stalls on DVE.
  - Cell+fraction via the fp32 magic-number trick: fl = (u+2^23)-2^23 is
    round-to-nearest(u); with u = pt*1024+1024 (cell-center offset), t=u-fl
    is the centered fraction.  2 DVE ops/coordinate vs 8 in v1.
  - qi double-buffered per chunk so the DVE's index write for iteration i+1
    never waits on the Pool engine still reading iteration i's indices.
"""
import numpy as np

H = W = 2048
NPTS = 262144
N_CORES = 8
PTS_PER_CORE = NPTS // N_CORES          # 32768
P = 128
F = PTS_PER_CORE // P                   # 256 free elems per partition
PAD = 2050                              # padded table row length
NCHUNK = 4
FC = F // NCHUNK                        # 64
MAGIC = 8388608.0                       # 2**23

_compiled = {}


def _build_nc(niter: int):
    import concourse.bass as bass
    import concourse.mybir as mybir
    import concourse.tile as tile
    from concourse import bacc

    f32 = mybir.dt.float32
    i32 = mybir.dt.int32
    Alu = mybir.AluOpType
    AF = mybir.ActivationFunctionType

    nc = bacc.Bacc("TRN2", target_bir_lowering=False, debug=False,
                   num_devices=N_CORES)
    tab = nc.dram_tensor("tab", [PAD * PAD, 8], f32, kind="ExternalInput").ap()
    p0x = nc.dram_tensor("p0x", [P, F], f32, kind="ExternalInput").ap()
    p0y = nc.dram_tensor("p0y", [P, F], f32, kind="ExternalInput").ap()
    outx = nc.dram_tensor("outx", [P, F], f32, kind="ExternalOutput").ap()
    outy = nc.dram_tensor("outy", [P, F], f32, kind="ExternalOutput").ap()

    with tile.TileContext(nc) as tc:
        with (
            tc.tile_pool(name="state", bufs=1) as state,
            tc.tile_pool(name="upool", bufs=3) as upool,
            tc.tile_pool(name="gbuf", bufs=3) as gbuf,
            tc.tile_pool(name="lpool", bufs=3) as lpool,
        ):
            px = state.tile([P, F], f32, tag="px")
            py = state.tile([P, F], f32, tag="py")
            nc.gpsimd.dma_start(out=px[:], in_=p0x[:])
            nc.gpsimd.dma_start(out=py[:], in_=p0y[:])

            txs = [state.tile([P, FC], f32, tag=f"tx{c}", name=f"tx{c}")
                   for c in range(NCHUNK)]
            tys = [state.tile([P, FC], f32, tag=f"ty{c}", name=f"ty{c}")
                   for c in range(NCHUNK)]
            qis = [[state.tile([P, FC], i32, tag=f"qi{c}_{b}", name=f"qi{c}_{b}")
                    for b in (0, 1)] for c in range(NCHUNK)]

            def coordq(c, it):
                cs = slice(c * FC, (c + 1) * FC)
                u2x = upool.tile([P, FC], f32, tag="u2x")
                u2y = upool.tile([P, FC], f32, tag="u2y")
                flx = upool.tile([P, FC], f32, tag="flx")
                fly = upool.tile([P, FC], f32, tag="fly")
                qf = upool.tile([P, FC], f32, tag="qf")
                # u2 = pt*1024 + 1024  (= padded x minus 0.5); Act engine
                nc.scalar.activation(out=u2x[:], in_=px[:, cs], func=AF.Copy,
                                     scale=1024.0, bias=1024.0)
                nc.scalar.activation(out=u2y[:], in_=py[:, cs], func=AF.Copy,
                                     scale=1024.0, bias=1024.0)
                # fl = rne(u2) via magic number; t = u2 - fl in [-0.5, 0.5]
                nc.vector.tensor_scalar(out=flx[:], in0=u2x[:], scalar1=MAGIC,
                                        scalar2=MAGIC, op0=Alu.add,
                                        op1=Alu.subtract)
                nc.vector.tensor_scalar(out=fly[:], in0=u2y[:], scalar1=MAGIC,
                                        scalar2=MAGIC, op0=Alu.add,
                                        op1=Alu.subtract)
                nc.vector.tensor_tensor(out=txs[c][:], in0=u2x[:], in1=flx[:],
                                        op=Alu.subtract)
                nc.vector.tensor_tensor(out=tys[c][:], in0=u2y[:], in1=fly[:],
                                        op=Alu.subtract)
                # q = fly*2050 + flx (exact in fp32; < 2^23)
                nc.vector.tensor_scalar(out=qf[:], in0=fly[:], scalar1=2050.0,
                                        scalar2=None, op0=Alu.mult)
                nc.vector.tensor_tensor(out=qf[:], in0=qf[:], in1=flx[:],
                                        op=Alu.add)
                nc.vector.tensor_copy(out=qis[c][it % 2][:], in_=qf[:])

            def gather(c, it):
                qi = qis[c][it % 2]
                g = gbuf.tile([P, FC, 8], f32, tag="g")
                for j in range(FC):
                    nc.gpsimd.indirect_dma_start(
                        out=g[:, j, :],
                        out_offset=None,
                        in_=tab[:, :],
                        in_offset=bass.IndirectOffsetOnAxis(
                            ap=qi[:, j:j + 1], axis=0),
                    )
                return g

            def lerpupd(c, g):
                cs = slice(c * FC, (c + 1) * FC)
                tx2 = txs[c][:].to_broadcast([P, FC, 2])
                ty2 = tys[c][:].to_broadcast([P, FC, 2])
                r1 = lpool.tile([P, FC, 2], f32, tag="r1")
                r2 = lpool.tile([P, FC, 2], f32, tag="r2")
                # val = (A + tx*B) + ty*(C + tx*D); slots [A_a,A_b,B_a,B_b,
                #                                         C_a,C_b,D_a,D_b]
                nc.vector.tensor_tensor(out=r1[:], in0=g[:, :, 2:4], in1=tx2,
                                        op=Alu.mult)
                nc.vector.tensor_tensor(out=r1[:], in0=r1[:], in1=g[:, :, 0:2],
                                        op=Alu.add)
                nc.vector.tensor_tensor(out=r2[:], in0=g[:, :, 6:8], in1=tx2,
                                        op=Alu.mult)
                nc.vector.tensor_tensor(out=r2[:], in0=r2[:], in1=g[:, :, 4:6],
                                        op=Alu.add)
                nc.vector.tensor_tensor(out=r2[:], in0=r2[:], in1=ty2,
                                        op=Alu.mult)
                nc.vector.tensor_tensor(out=r1[:], in0=r1[:], in1=r2[:],
                                        op=Alu.add)
                # pt += s ; clip to [-1, 1]
                nc.vector.tensor_tensor(out=px[:, cs], in0=px[:, cs],
                                        in1=r1[:, :, 0], op=Alu.add)
                nc.vector.tensor_tensor(out=py[:, cs], in0=py[:, cs],
                                        in1=r1[:, :, 1], op=Alu.add)
                nc.vector.tensor_scalar(out=px[:, cs], in0=px[:, cs],
                                        scalar1=-1.0, scalar2=1.0,
                                        op0=Alu.max, op1=Alu.min)
                nc.vector.tensor_scalar(out=py[:, cs], in0=py[:, cs],
                                        scalar1=-1.0, scalar2=1.0,
                                        op0=Alu.max, op1=Alu.min)

            for c in range(NCHUNK):
                coordq(c, 0)
            for it in range(niter):
                for c in range(NCHUNK):
                    g = gather(c, it)
                    lerpupd(c, g)
                    if it + 1 < niter:
                        coordq(c, it + 1)

            # final: pix = (pt + 1) * 1023.5 on the Act engine
            ox = state.tile([P, F], f32, tag="ox")
            oy = state.tile([P, F], f32, tag="oy")
            nc.scalar.activation(out=ox[:], in_=px[:], func=mybir.ActivationFunctionType.Copy,
                                 scale=1023.5, bias=1023.5)
            nc.scalar.activation(out=oy[:], in_=py[:], func=mybir.ActivationFunctionType.Copy,
                                 scale=1023.5, bias=1023.5)
            nc.gpsimd.dma_start(out=outx[:], in_=ox[:])
            nc.gpsimd.dma_start(out=outy[:], in_=oy[:])

    nc.compile()
    return nc


def _build_table(dP: np.ndarray) -> np.ndarray:
    """T[r*2050+c] = centered-bilinear coeffs [A,B,C,D] x [a,b] channels."""
    scale = np.float32(2.0 / 2047.0)
    ima = (dP[1] * scale).astype(np.float32)   # adds to pt x
    imb = (dP[0] * scale).astype(np.float32)   # adds to pt y
    imp = np.zeros((PAD + 1, PAD + 1, 2), np.float32)
    imp[1:H + 1, 1:W + 1, 0] = ima
    imp[1:H + 1, 1:W + 1, 1] = imb
    t00 = imp[:PAD, :PAD]
    t01 = imp[:PAD, 1:]
    t10 = imp[1:, :PAD]
    t11 = imp[1:, 1:]
    T = np.empty((PAD, PAD, 8), np.float32)
    T[:, :, 0:2] = (t00 + t01 + t10 + t11) * np.float32(0.25)       # A
    T[:, :, 2:4] = ((t01 - t00) + (t11 - t10)) * np.float32(0.5)    # B
    T[:, :, 4:6] = ((t10 - t00) + (t11 - t01)) * np.float32(0.5)    # C
    T[:, :, 6:8] = (t11 - t10) - (t01 - t00)                        # D
    return T.reshape(PAD * PAD, 8)


def _initial_pts(inds: np.ndarray):
    f = np.float32
    sizes = f(2047.0)
    ptx = inds[1].astype(f) / sizes * f(2.0) - f(1.0)
    pty = inds[0].astype(f) / sizes * f(2.0) - f(1.0)
    return ptx, pty


def kernel(dP: np.ndarray, inds: np.ndarray, niter) -> np.ndarray:
    from concourse.bass_utils import run_bass_kernel_spmd

    niter = int(niter)
    dP = np.asarray(dP, np.float32)
    inds = np.asarray(inds)

    if niter not in _compiled:
        _compiled[niter] = _build_nc(niter)
    nc = _compiled[niter]

    T = _build_table(dP)
    ptx, pty = _initial_pts(inds)

    in_maps = []
    for i in range(N_CORES):
        sl = slice(i * PTS_PER_CORE, (i + 1) * PTS_PER_CORE)
        in_maps.append({
            "tab": T,
            "p0x": ptx[sl].reshape(P, F),
            "p0y": pty[sl].reshape(P, F),
        })

    res = run_bass_kernel_spmd(nc, in_maps, list(range(N_CORES)))

    out = np.empty((2, NPTS), np.float32)
    for i in range(N_CORES):
        sl = slice(i * PTS_PER_CORE, (i + 1) * PTS_PER_CORE)
        out[0, sl] = res.results[i]["outy"].reshape(-1)
        out[1, sl] = res.results[i]["outx"].reshape(-1)
    return out
